# revision 17
# baseline (speedup 1.0000x reference)
"""Multi-head attention (RoPE, causal) Trainium2 Bass kernel, 8-way sharded.

Sharding: tensor-parallel over heads x data-parallel over batch.
  core c (0..7): batch b = c // 4, head group hg = c % 4 -> heads [4*hg, 4*hg+4).
Each core computes its 4 heads' QKV projection, RoPE, causal attention, and a
partial output projection (its 512 columns of the E-dim contraction).  The host
sums the 4 partials per batch and transposes back.

Device-side layouts are transposed ([feature, seq]) so matmuls feed the PE
array directly; fp32 data is run through the PE in float32r (TF32-like) mode,
which is full rate for moving dims >= 256.  Softmax skips the max-subtraction
(logits are O(+-10) here so exp cannot overflow) which lets everything stay in
the [key, query] orientation: the denominator is a ones-vector matmul and the
1/denom broadcast across partitions is a K=1 ones matmul.
"""

import sys

sys.path.insert(0, "/opt/trn_rl_repo")

import numpy as np

import concourse.bass as bass  # noqa: F401
import concourse.tile as tile
from concourse import bacc, mybir
from concourse import bass_utils

# Problem shape (hardcoded per contract).
B = 2
S = 2048
E = 2048
H = 16
D = 128
N_CORES = 8
GPB = N_CORES // B  # head groups per batch = 4
HPC = H // GPB  # heads per core = 4
DPC = HPC * D  # feature cols per core = 512
SBLK = 512
NSBLK = S // SBLK  # 4
NECH = E // 128  # 16 contraction chunks
NST = S // 128  # 16 seq tiles
SM_SCALE = float(D) ** -0.5

F32 = mybir.dt.float32
F32R = mybir.dt.float32r

_CACHE = {}
_RUN_KWARGS = {}


def _build_nc():
    nc = bacc.Bacc(
        "TRN2",
        target_bir_lowering=False,
        debug=False,
        enable_asserts=True,
        num_devices=N_CORES,
    )
    xT = nc.dram_tensor("xT", [E, S], F32R, kind="ExternalInput").ap()
    wqkT = nc.dram_tensor("wqkT", [E, 2 * DPC], F32R, kind="ExternalInput").ap()
    wvT = nc.dram_tensor("wvT", [E, DPC], F32R, kind="ExternalInput").ap()
    woutT = nc.dram_tensor("woutT", [DPC, E], F32R, kind="ExternalInput").ap()
    cosT = nc.dram_tensor("cosT", [D, S], F32, kind="ExternalInput").ap()
    sinTs = nc.dram_tensor("sinTs", [D, S], F32, kind="ExternalInput").ap()
    bmask = nc.dram_tensor("bmask", [128, 4 * SBLK], F32, kind="ExternalInput").ap()
    ones = nc.dram_tensor("ones", [128, 128], F32R, kind="ExternalInput").ap()
    outT = nc.dram_tensor("outT", [E, S], F32, kind="ExternalOutput").ap()

    with tile.TileContext(nc) as tc, nc.allow_low_precision(reason="fp32r matmuls"):
        with (
            tc.tile_pool(name="const", bufs=1) as const_pool,
            tc.tile_pool(name="vres", bufs=1) as v_pool,
            tc.tile_pool(name="dram", bufs=1, space="DRAM") as dram_pool,
        ):
            ones_sb = const_pool.tile([128, 128], F32R)
            nc.sync.dma_start(ones_sb[:], ones[:])

            # q/k spill, one DRAM tile per (head, s-block) for fine-grained deps
            qsp = [
                [dram_pool.tile([128, SBLK], F32R, name=f"qsp{h}_{sb}") for sb in range(NSBLK)]
                for h in range(HPC)
            ]
            ksp = [
                [dram_pool.tile([128, SBLK], F32R, name=f"ksp{h}_{sb}") for sb in range(NSBLK)]
                for h in range(HPC)
            ]

            # ---------- Phase 1: QKV projection + RoPE (q/k spill to DRAM) ----------
            v_sb = v_pool.tile([128, NST * DPC], F32R)
            with (
                tc.tile_pool(name="rope_c", bufs=1) as rope_const,
                tc.tile_pool(name="rtmp", bufs=2) as rtmp_pool,
                tc.tile_pool(name="rout", bufs=3) as rout_pool,
                tc.tile_pool(name="ps_qk", bufs=3, space="PSUM") as psqk_pool,
                tc.tile_pool(name="ps_v", bufs=2, space="PSUM") as psv_pool,
            ):
                cos_sb = rope_const.tile([D, S], F32)
                sin_sb = rope_const.tile([D, S], F32)

                def rope_spill(ps, dst, ssl):
                    # dst: per-(head, s-block) DRAM tile [128, SBLK]
                    t1 = rtmp_pool.tile([128, SBLK], F32, tag="t1", name="t1")
                    nc.vector.tensor_mul(t1[:], ps[:], cos_sb[:, ssl])
                    t2 = rtmp_pool.tile([128, SBLK], F32, tag="t2", name="t2")
                    nc.vector.tensor_mul(t2[0:64, :], ps[64:128, :], sin_sb[0:64, ssl])
                    nc.vector.tensor_mul(
                        t2[64:128, :], ps[0:64, :], sin_sb[64:128, ssl]
                    )
                    ro = rout_pool.tile([128, SBLK], F32R, tag="ro", name="ro")
                    nc.vector.tensor_add(ro[:], t1[:], t2[:])
                    nc.sync.dma_start(dst[:], ro[:])

                # ----- single pass: q, k heads + v (all weights resident) -----
                with (
                    tc.tile_pool(name="wq", bufs=NECH) as wq_pool,
                    tc.tile_pool(name="wk", bufs=NECH) as wk_pool,
                    tc.tile_pool(name="wv", bufs=NECH) as wv_pool,
                    tc.tile_pool(name="xts", bufs=24) as xts_pool,
                ):
                    # interleave wq + first s-block xt loads so the first
                    # accumulation chain's deps land earliest
                    wq_t = []
                    xts0 = []
                    for e in range(NECH):
                        wt = wq_pool.tile([128, DPC], F32R, tag="wq", name="wq")
                        nc.sync.dma_start(wt[:], wqkT[e * 128 : (e + 1) * 128, 0:DPC])
                        wq_t.append(wt)
                        xt = xts_pool.tile([128, SBLK], F32R, tag="xt", name="xt")
                        nc.sync.dma_start(xt[:], xT[e * 128 : (e + 1) * 128, 0:SBLK])
                        xts0.append(xt)
                    wk_t = []
                    for e in range(NECH):
                        wt = wk_pool.tile([128, DPC], F32R, tag="wk", name="wk")
                        nc.sync.dma_start(
                            wt[:], wqkT[e * 128 : (e + 1) * 128, DPC : 2 * DPC]
                        )
                        wk_t.append(wt)
                    nc.sync.dma_start(cos_sb[:], cosT[:])
                    nc.sync.dma_start(sin_sb[:], sinTs[:])
                    wv_t = []
                    for e in range(NECH):
                        wt = wv_pool.tile([128, DPC], F32R, tag="wv", name="wv")
                        nc.sync.dma_start(wt[:], wvT[e * 128 : (e + 1) * 128, :])
                        wv_t.append(wt)
                    for sb in range(NSBLK):
                        ssl = slice(sb * SBLK, (sb + 1) * SBLK)
                        if sb == 0:
                            xts = xts0
                        else:
                            xts = []
                            for e in range(NECH):
                                xt = xts_pool.tile([128, SBLK], F32R, tag="xt", name="xt")
                                nc.sync.dma_start(xt[:], xT[e * 128 : (e + 1) * 128, ssl])
                                xts.append(xt)
                        for m in range(HPC):
                            ps = psqk_pool.tile([128, SBLK], F32, name="psq")
                            for e in range(NECH):
                                nc.tensor.matmul(
                                    ps[:],
                                    wq_t[e][:, m * 128 : (m + 1) * 128],
                                    xts[e][:],
                                    start=(e == 0),
                                    stop=(e == NECH - 1),
                                )
                            rope_spill(ps, qsp[m][sb], ssl)
                        for m in range(HPC):
                            ps = psqk_pool.tile([128, SBLK], F32, name="psk")
                            for e in range(NECH):
                                nc.tensor.matmul(
                                    ps[:],
                                    wk_t[e][:, m * 128 : (m + 1) * 128],
                                    xts[e][:],
                                    start=(e == 0),
                                    stop=(e == NECH - 1),
                                )
                            rope_spill(ps, ksp[m][sb], ssl)
                        # v: natural layout [s, d']
                        for sm in range(SBLK // 128):
                            st = sb * (SBLK // 128) + sm
                            psv = psv_pool.tile([128, DPC], F32, name="psv")
                            for e in range(NECH):
                                nc.tensor.matmul(
                                    psv[:],
                                    xts[e][:, sm * 128 : (sm + 1) * 128],
                                    wv_t[e][:],
                                    start=(e == 0),
                                    stop=(e == NECH - 1),
                                )
                            nc.scalar.copy(v_sb[:, st * DPC : (st + 1) * DPC], psv[:])

            # ---------------- Phase 2: attention per head ----------------
            # late-entered pools (avoid reserving SBUF during phase 1)
            ctx_cm = tc.tile_pool(name="ctx", bufs=1)
            ctx_pool = ctx_cm.__enter__()
            wo_cm = tc.tile_pool(name="wo", bufs=HPC)
            wo_pool = wo_cm.__enter__()
            lconst_cm = tc.tile_pool(name="lconst", bufs=1)
            lconst_pool = lconst_cm.__enter__()
            bm_sb = lconst_pool.tile([128, 4 * SBLK], F32, name="bm_sb")
            nc.sync.dma_start(bm_sb[:], bmask[:])
            wo_t = []
            for h in range(HPC):
                wt = wo_pool.tile([128, E], F32R, tag="wo", name="wo")
                nc.sync.dma_start(wt[:], woutT[h * 128 : (h + 1) * 128, :])
                wo_t.append(wt)
            ctx16 = [[None] * NSBLK for _ in range(HPC)]
            with (
                tc.tile_pool(name="qk", bufs=2) as qk_pool,
                tc.tile_pool(name="exp", bufs=4) as exp_pool,
                tc.tile_pool(name="rcp", bufs=2) as rcp_pool,
                tc.tile_pool(name="bcs", bufs=2) as bcs_pool,
                tc.tile_pool(name="ps_sc", bufs=4, space="PSUM") as sc_pool,
                tc.tile_pool(name="ps_ctx", bufs=2, space="PSUM") as pctx_pool,
                tc.tile_pool(name="ps_den", bufs=2, space="PSUM") as den_pool,
            ):
                for h in range(HPC):
                    for t in range(NSBLK):
                        ctx16[h][t] = ctx_pool.tile(
                            [128, SBLK], F32R, tag=f"ctx{h}_{t}", name=f"ctx{h}_{t}"
                        )
                for h in range(HPC):
                    qh_t = []
                    kh_t = []
                    for sb in range(NSBLK):
                        qt = qk_pool.tile([128, SBLK], F32R, tag=f"qh{sb}", name="qh")
                        nc.sync.dma_start(qt[:], qsp[h][sb][:])
                        qh_t.append(qt)
                        kt = qk_pool.tile([128, SBLK], F32R, tag=f"kh{sb}", name="kh")
                        nc.sync.dma_start(kt[:], ksp[h][sb][:])
                        kh_t.append(kt)
                    for t in range(NSBLK):
                        njt = 4 * (t + 1)
                        ctx_ps = pctx_pool.tile([128, SBLK], F32)
                        den_ps = den_pool.tile([1, SBLK], F32)

                        def emit_denctx(work):
                            jt, lo, ex = work
                            nc.tensor.matmul(
                                den_ps[0:1, lo:SBLK],
                                ones_sb[:, 0:1],
                                ex[:, lo:SBLK],
                                start=(jt == 0),
                                stop=(jt == njt - 1),
                                skip_group_check=True,
                            )
                            nc.tensor.matmul(
                                ctx_ps[:, lo:SBLK],
                                v_sb[
                                    :, jt * DPC + h * 128 : jt * DPC + (h + 1) * 128
                                ],
                                ex[:, lo:SBLK],
                                start=(jt == 0),
                                stop=(jt == njt - 1),
                                skip_group_check=True,
                            )

                        inflight = []
                        for jt in range(njt):
                            o = jt - 4 * t
                            # causal: columns i < jt*128 of this i-block are
                            # fully masked for this j-tile -> shrink width
                            lo = max(o, 0) * 128
                            sc_ps = sc_pool.tile([128, SBLK], F32)
                            nc.tensor.matmul(
                                sc_ps[:, lo:SBLK],
                                kh_t[jt // 4][:, (jt % 4) * 128 : (jt % 4 + 1) * 128],
                                qh_t[t][:, lo:SBLK],
                                start=True,
                                stop=True,
                            )
                            ex = exp_pool.tile([128, SBLK], F32R, tag="ex")
                            nc.scalar.activation(
                                ex[:, lo:SBLK],
                                sc_ps[:, lo:SBLK],
                                mybir.ActivationFunctionType.Exp,
                                scale=SM_SCALE,
                            )
                            if o >= 0:
                                nc.gpsimd.tensor_mul(
                                    ex[:, lo:SBLK],
                                    ex[:, lo:SBLK],
                                    bm_sb[:, o * SBLK + lo : (o + 1) * SBLK],
                                )
                            inflight.append((jt, lo, ex))
                            if len(inflight) > 2:
                                emit_denctx(inflight.pop(0))
                        for work in inflight:
                            emit_denctx(work)
                        _finalize(nc, (h, t, ctx_ps, den_ps), rcp_pool,
                                  bcs_pool, ctx16)

            # ---------------- Phase 3: output projection ----------------
            with (
                tc.tile_pool(name="osb", bufs=4) as osb_pool,
                tc.tile_pool(name="ps_o", bufs=2, space="PSUM") as pso_pool,
            ):
                for sb in range(NSBLK):
                    ssl = slice(sb * SBLK, (sb + 1) * SBLK)
                    for m in range(E // 128):
                        po = pso_pool.tile([128, SBLK], F32, name="po")
                        for h in range(HPC):
                            nc.tensor.matmul(
                                po[:],
                                wo_t[h][:, m * 128 : (m + 1) * 128],
                                ctx16[h][sb][:],
                                start=(h == 0),
                                stop=(h == HPC - 1),
                            )
                        ot = osb_pool.tile([128, SBLK], F32, tag="ot", name="ot")
                        nc.scalar.copy(ot[:], po[:])
                        nc.sync.dma_start(outT[m * 128 : (m + 1) * 128, ssl], ot[:])
            lconst_cm.__exit__(None, None, None)
            wo_cm.__exit__(None, None, None)
            ctx_cm.__exit__(None, None, None)

    nc.compile()
    return nc


def _finalize(nc, pending, rcp_pool, bcs_pool, ctx16):
    h, t, ctx_ps, den_ps = pending
    rc = rcp_pool.tile([1, SBLK], F32, tag="rc", name="rc")
    nc.vector.reciprocal_approx_fast(out=rc[0:1, :], in_=den_ps[0:1, :])
    bc_sb = bcs_pool.tile([128, SBLK], F32, tag="bc", name="bc_sb")
    nc.gpsimd.partition_broadcast(bc_sb[:], rc[0:1, :])
    nc.vector.tensor_mul(ctx16[h][t][:], ctx_ps[:], bc_sb[:])


def _rope_tables():
    inv_freq = 1.0 / (10000.0 ** (np.arange(0, D, 2, dtype=np.float64) / D))
    t = np.arange(S, dtype=np.float64)
    freqs = np.outer(t, inv_freq)  # (S, D/2)
    emb = np.concatenate([freqs, freqs], axis=-1)  # (S, D)
    cosT = np.cos(emb).T.astype(np.float32).copy()  # (D, S)
    sinT = np.sin(emb).T.astype(np.float32)
    sinTs = sinT.copy()
    sinTs[: D // 2] = -sinT[: D // 2]
    return cosT, np.ascontiguousarray(sinTs)


def _binmask():
    r = np.arange(128)[:, None]
    c = np.arange(SBLK)[None, :]
    blocks = [(r + o * 128 <= c).astype(np.float32) for o in range(4)]
    return np.ascontiguousarray(np.concatenate(blocks, axis=1))


def _numpy_fallback(x, mask, wqkv, bqkv, wout, bout):
    qkv = x @ wqkv.T + bqkv
    q, k, v = np.split(qkv, 3, axis=-1)
    q = q.reshape(B, S, H, D).transpose(0, 2, 1, 3)
    k = k.reshape(B, S, H, D).transpose(0, 2, 1, 3)
    v = v.reshape(B, S, H, D).transpose(0, 2, 1, 3)
    inv_freq = 1.0 / (10000.0 ** (np.arange(0, D, 2, dtype=np.float32) / D))
    t = np.arange(S, dtype=np.float32)
    freqs = np.outer(t, inv_freq)
    emb = np.concatenate([freqs, freqs], axis=-1)
    cos, sin = np.cos(emb), np.sin(emb)

    def rot(a):
        a1, a2 = np.split(a, 2, axis=-1)
        return np.concatenate([-a2, a1], axis=-1)

    q = q * cos + rot(q) * sin
    k = k * cos + rot(k) * sin
    scores = np.einsum("bhqd,bhkd->bhqk", q, k) * SM_SCALE
    scores = np.where(mask, -np.inf, scores)
    scores = scores - scores.max(axis=-1, keepdims=True)
    w = np.exp(scores)
    w = w / w.sum(axis=-1, keepdims=True)
    ctx = np.einsum("bhqk,bhkd->bhqd", w, v)
    ctx = ctx.transpose(0, 2, 1, 3).reshape(B, S, E)
    return (ctx @ wout.T + bout).astype(np.float32)


def kernel(x, mask, wqkv, bqkv, wout, bout, **_):
    x = np.ascontiguousarray(np.asarray(x), dtype=np.float32)
    wqkv = np.ascontiguousarray(np.asarray(wqkv), dtype=np.float32)
    bqkv = np.asarray(bqkv, dtype=np.float32)
    wout = np.ascontiguousarray(np.asarray(wout), dtype=np.float32)
    bout = np.asarray(bout, dtype=np.float32)
    mask = np.asarray(mask)

    causal = np.array_equal(mask, np.triu(np.ones((S, S), dtype=bool), k=1))
    if not causal or np.any(bqkv):
        return _numpy_fallback(x, mask, wqkv, bqkv, wout, bout)

    if "nc" not in _CACHE:
        _CACHE["nc"] = _build_nc()
    nc = _CACHE["nc"]

    cosT, sinTs = _rope_tables()
    bm = _binmask()
    ones = np.ones((128, 128), dtype=np.float32)

    in_maps = []
    for c in range(N_CORES):
        b, hg = divmod(c, GPB)
        cols = slice(hg * DPC, (hg + 1) * DPC)
        wq = wqkv[0 * E : 1 * E, :][cols, :]  # (512, E)
        wk = wqkv[1 * E : 2 * E, :][cols, :]
        wv = wqkv[2 * E : 3 * E, :][cols, :]
        in_maps.append(
            {
                "xT": np.ascontiguousarray(x[b].T),
                "wqkT": np.ascontiguousarray(np.concatenate([wq, wk], axis=0).T),
                "wvT": np.ascontiguousarray(wv.T),
                "woutT": np.ascontiguousarray(wout[:, cols].T),  # (512, E)
                "cosT": cosT,
                "sinTs": sinTs,
                "bmask": bm,
                "ones": ones,
            }
        )

    res = bass_utils.run_bass_kernel_spmd(
        nc, in_maps, core_ids=list(range(N_CORES)), **_RUN_KWARGS
    )
    _CACHE["last_results"] = res

    out = np.empty((B, S, E), dtype=np.float32)
    for b in range(B):
        acc = res.results[b * GPB]["outT"].copy()
        for g in range(1, GPB):
            acc += res.results[b * GPB + g]["outT"]
        out[b] = acc.T
    out += bout
    return out


# revision 20
# speedup vs baseline: 1.4432x; 1.4432x over previous
"""Multi-head attention (RoPE, causal) Trainium2 Bass kernel, 8-way sharded.

Sharding: tensor-parallel over heads x data-parallel over batch.
  core c (0..7): batch b = c // 4, head group hg = c % 4 -> heads [4*hg, 4*hg+4).
Each core computes its 4 heads' QKV projection, RoPE, causal attention, and a
partial output projection (its 512 columns of the E-dim contraction).  The host
sums the 4 partials per batch and transposes back.

Device-side layouts are transposed ([feature, seq]) so matmuls feed the PE
array directly; fp32 data is run through the PE in float32r (TF32-like) mode,
which is full rate for moving dims >= 256.  Softmax skips the max-subtraction
(logits are O(+-10) here so exp cannot overflow) which lets everything stay in
the [key, query] orientation: the denominator is a ones-vector matmul and the
1/denom broadcast across partitions is a K=1 ones matmul.
"""

import sys

sys.path.insert(0, "/opt/trn_rl_repo")

import numpy as np

import concourse.bass as bass  # noqa: F401
import concourse.tile as tile
from concourse import bacc, mybir
from concourse import bass_utils

# Problem shape (hardcoded per contract).
B = 2
S = 2048
E = 2048
H = 16
D = 128
N_CORES = 8
GPB = N_CORES // B  # head groups per batch = 4
HPC = H // GPB  # heads per core = 4
DPC = HPC * D  # feature cols per core = 512
SBLK = 512
NSBLK = S // SBLK  # 4
NECH = E // 128  # 16 contraction chunks
NST = S // 128  # 16 seq tiles
SM_SCALE = float(D) ** -0.5

F32 = mybir.dt.float32
F32R = mybir.dt.float32r

_CACHE = {}
_RUN_KWARGS = {}


def _build_nc():
    nc = bacc.Bacc(
        "TRN2",
        target_bir_lowering=False,
        debug=False,
        enable_asserts=True,
        num_devices=N_CORES,
    )
    xT = nc.dram_tensor("xT", [E, S], F32R, kind="ExternalInput").ap()
    wqkT = nc.dram_tensor("wqkT", [E, 2 * DPC], F32R, kind="ExternalInput").ap()
    wvT = nc.dram_tensor("wvT", [E, DPC], F32R, kind="ExternalInput").ap()
    woutT = nc.dram_tensor("woutT", [DPC, E], F32R, kind="ExternalInput").ap()
    cosT = nc.dram_tensor("cosT", [D, S], F32, kind="ExternalInput").ap()
    sinTs = nc.dram_tensor("sinTs", [D, S], F32, kind="ExternalInput").ap()
    bmask = nc.dram_tensor("bmask", [128, 4 * SBLK], F32, kind="ExternalInput").ap()
    ones = nc.dram_tensor("ones", [128, 128], F32R, kind="ExternalInput").ap()
    outT = nc.dram_tensor("outT", [E, S], F32, kind="ExternalOutput").ap()

    with tile.TileContext(nc) as tc, nc.allow_low_precision(reason="fp32r matmuls"):
        with (
            tc.tile_pool(name="const", bufs=1) as const_pool,
            tc.tile_pool(name="vres", bufs=1) as v_pool,
            tc.tile_pool(name="dram", bufs=1, space="DRAM") as dram_pool,
        ):
            ones_sb = const_pool.tile([128, 128], F32R)
            nc.sync.dma_start(ones_sb[:], ones[:])

            # q/k spill, one DRAM tile per (head, s-block) for fine-grained deps
            qsp = [
                [dram_pool.tile([128, SBLK], F32R, name=f"qsp{h}_{sb}") for sb in range(NSBLK)]
                for h in range(HPC)
            ]
            ksp = [
                [dram_pool.tile([128, SBLK], F32R, name=f"ksp{h}_{sb}") for sb in range(NSBLK)]
                for h in range(HPC)
            ]

            # ---------- Phase 1: QKV projection + RoPE (q/k spill to DRAM) ----------
            v_sb = v_pool.tile([128, NST * DPC], F32R)
            with (
                tc.tile_pool(name="rope_c", bufs=1) as rope_const,
                tc.tile_pool(name="rtmp", bufs=2) as rtmp_pool,
                tc.tile_pool(name="rout", bufs=3) as rout_pool,
                tc.tile_pool(name="ps_qk", bufs=3, space="PSUM") as psqk_pool,
                tc.tile_pool(name="ps_v", bufs=2, space="PSUM") as psv_pool,
            ):
                cos_sb = rope_const.tile([D, S], F32)
                sin_sb = rope_const.tile([D, S], F32)

                def rope_spill(ps, dst, ssl):
                    # dst: per-(head, s-block) DRAM tile [128, SBLK]
                    t1 = rtmp_pool.tile([128, SBLK], F32, tag="t1", name="t1")
                    nc.vector.tensor_mul(t1[:], ps[:], cos_sb[:, ssl])
                    t2 = rtmp_pool.tile([128, SBLK], F32, tag="t2", name="t2")
                    nc.vector.tensor_mul(t2[0:64, :], ps[64:128, :], sin_sb[0:64, ssl])
                    nc.vector.tensor_mul(
                        t2[64:128, :], ps[0:64, :], sin_sb[64:128, ssl]
                    )
                    ro = rout_pool.tile([128, SBLK], F32R, tag="ro", name="ro")
                    nc.vector.tensor_add(ro[:], t1[:], t2[:])
                    nc.sync.dma_start(dst[:], ro[:])

                # ----- single pass: q, k heads + v (all weights resident) -----
                with (
                    tc.tile_pool(name="wq", bufs=NECH) as wq_pool,
                    tc.tile_pool(name="wk", bufs=NECH) as wk_pool,
                    tc.tile_pool(name="wv", bufs=NECH) as wv_pool,
                    tc.tile_pool(name="xts", bufs=24) as xts_pool,
                ):
                    # interleave wq + first s-block xt loads so the first
                    # accumulation chain's deps land earliest
                    wq_t = []
                    xts0 = []
                    for e in range(NECH):
                        wt = wq_pool.tile([128, DPC], F32R, tag="wq", name="wq")
                        nc.sync.dma_start(wt[:], wqkT[e * 128 : (e + 1) * 128, 0:DPC])
                        wq_t.append(wt)
                        xt = xts_pool.tile([128, SBLK], F32R, tag="xt", name="xt")
                        nc.sync.dma_start(xt[:], xT[e * 128 : (e + 1) * 128, 0:SBLK])
                        xts0.append(xt)
                    wk_t = []
                    for e in range(NECH):
                        wt = wk_pool.tile([128, DPC], F32R, tag="wk", name="wk")
                        nc.sync.dma_start(
                            wt[:], wqkT[e * 128 : (e + 1) * 128, DPC : 2 * DPC]
                        )
                        wk_t.append(wt)
                    wv_t = []
                    for e in range(NECH):
                        wt = wv_pool.tile([128, DPC], F32R, tag="wv", name="wv")
                        nc.sync.dma_start(wt[:], wvT[e * 128 : (e + 1) * 128, :])
                        wv_t.append(wt)
                    nc.sync.dma_start(cos_sb[:], cosT[:])
                    nc.sync.dma_start(sin_sb[:], sinTs[:])
                    for sb in range(NSBLK):
                        ssl = slice(sb * SBLK, (sb + 1) * SBLK)
                        if sb == 0:
                            xts = xts0
                        else:
                            xts = []
                            for e in range(NECH):
                                xt = xts_pool.tile([128, SBLK], F32R, tag="xt", name="xt")
                                nc.sync.dma_start(xt[:], xT[e * 128 : (e + 1) * 128, ssl])
                                xts.append(xt)
                        for m in range(HPC):
                            ps = psqk_pool.tile([128, SBLK], F32, tag="psqk", name="psq")
                            for e in range(NECH):
                                nc.tensor.matmul(
                                    ps[:],
                                    wq_t[e][:, m * 128 : (m + 1) * 128],
                                    xts[e][:],
                                    start=(e == 0),
                                    stop=(e == NECH - 1),
                                )
                            rope_spill(ps, qsp[m][sb], ssl)
                        for m in range(HPC):
                            ps = psqk_pool.tile([128, SBLK], F32, tag="psqk", name="psk")
                            for e in range(NECH):
                                nc.tensor.matmul(
                                    ps[:],
                                    wk_t[e][:, m * 128 : (m + 1) * 128],
                                    xts[e][:],
                                    start=(e == 0),
                                    stop=(e == NECH - 1),
                                )
                            rope_spill(ps, ksp[m][sb], ssl)
                        # v: natural layout [s, d']
                        for sm in range(SBLK // 128):
                            st = sb * (SBLK // 128) + sm
                            psv = psv_pool.tile([128, DPC], F32, name="psv")
                            for e in range(NECH):
                                nc.tensor.matmul(
                                    psv[:],
                                    xts[e][:, sm * 128 : (sm + 1) * 128],
                                    wv_t[e][:],
                                    start=(e == 0),
                                    stop=(e == NECH - 1),
                                )
                            nc.scalar.copy(v_sb[:, st * DPC : (st + 1) * DPC], psv[:])

            # ---------------- Phase 2: attention per head ----------------
            # late-entered pools (avoid reserving SBUF during phase 1)
            ctx_cm = tc.tile_pool(name="ctx", bufs=1)
            ctx_pool = ctx_cm.__enter__()
            wo_cm = tc.tile_pool(name="wo", bufs=HPC)
            wo_pool = wo_cm.__enter__()
            lconst_cm = tc.tile_pool(name="lconst", bufs=1)
            lconst_pool = lconst_cm.__enter__()
            bm_sb = lconst_pool.tile([128, 4 * SBLK], F32, name="bm_sb")
            nc.sync.dma_start(bm_sb[:], bmask[:])
            wo_t = []
            for h in range(HPC):
                wt = wo_pool.tile([128, E], F32R, tag="wo", name="wo")
                nc.sync.dma_start(wt[:], woutT[h * 128 : (h + 1) * 128, :])
                wo_t.append(wt)
            ctx16 = [[None] * NSBLK for _ in range(HPC)]
            with (
                tc.tile_pool(name="qk", bufs=2) as qk_pool,
                tc.tile_pool(name="exp", bufs=4) as exp_pool,
                tc.tile_pool(name="rcp", bufs=2) as rcp_pool,
                tc.tile_pool(name="bcs", bufs=2) as bcs_pool,
                tc.tile_pool(name="ps_sc", bufs=4, space="PSUM") as sc_pool,
                tc.tile_pool(name="ps_ctx", bufs=2, space="PSUM") as pctx_pool,
                tc.tile_pool(name="ps_den", bufs=2, space="PSUM") as den_pool,
            ):
                for h in range(HPC):
                    for t in range(NSBLK):
                        ctx16[h][t] = ctx_pool.tile(
                            [128, SBLK], F32R, tag=f"ctx{h}_{t}", name=f"ctx{h}_{t}"
                        )
                for h in range(HPC):
                    qh_t = []
                    kh_t = []
                    for sb in range(NSBLK):
                        qt = qk_pool.tile([128, SBLK], F32R, tag=f"qh{sb}", name="qh")
                        nc.sync.dma_start(qt[:], qsp[h][sb][:])
                        qh_t.append(qt)
                        kt = qk_pool.tile([128, SBLK], F32R, tag=f"kh{sb}", name="kh")
                        nc.sync.dma_start(kt[:], ksp[h][sb][:])
                        kh_t.append(kt)
                    for t in range(NSBLK):
                        njt = 4 * (t + 1)
                        ctx_ps = pctx_pool.tile([128, SBLK], F32)
                        den_ps = den_pool.tile([1, SBLK], F32)

                        def emit_denctx(work):
                            jt, lo, ex = work
                            nc.tensor.matmul(
                                den_ps[0:1, lo:SBLK],
                                ones_sb[:, 0:1],
                                ex[:, lo:SBLK],
                                start=(jt == 0),
                                stop=(jt == njt - 1),
                                skip_group_check=True,
                            )
                            nc.tensor.matmul(
                                ctx_ps[:, lo:SBLK],
                                v_sb[
                                    :, jt * DPC + h * 128 : jt * DPC + (h + 1) * 128
                                ],
                                ex[:, lo:SBLK],
                                start=(jt == 0),
                                stop=(jt == njt - 1),
                                skip_group_check=True,
                            )

                        inflight = []
                        for jt in range(njt):
                            o = jt - 4 * t
                            # causal: columns i < jt*128 of this i-block are
                            # fully masked for this j-tile -> shrink width
                            lo = max(o, 0) * 128
                            sc_ps = sc_pool.tile([128, SBLK], F32)
                            nc.tensor.matmul(
                                sc_ps[:, lo:SBLK],
                                kh_t[jt // 4][:, (jt % 4) * 128 : (jt % 4 + 1) * 128],
                                qh_t[t][:, lo:SBLK],
                                start=True,
                                stop=True,
                            )
                            ex = exp_pool.tile([128, SBLK], F32R, tag="ex")
                            nc.scalar.activation(
                                ex[:, lo:SBLK],
                                sc_ps[:, lo:SBLK],
                                mybir.ActivationFunctionType.Exp,
                                scale=SM_SCALE,
                            )
                            if o >= 0:
                                nc.vector.tensor_mul(
                                    ex[:, lo:SBLK],
                                    ex[:, lo:SBLK],
                                    bm_sb[:, o * SBLK + lo : (o + 1) * SBLK],
                                )
                            inflight.append((jt, lo, ex))
                            if len(inflight) > 2:
                                emit_denctx(inflight.pop(0))
                        for work in inflight:
                            emit_denctx(work)
                        _finalize(nc, (h, t, ctx_ps, den_ps), rcp_pool,
                                  bcs_pool, ctx16)

            # ---------------- Phase 3: output projection ----------------
            with (
                tc.tile_pool(name="osb", bufs=4) as osb_pool,
                tc.tile_pool(name="ps_o", bufs=2, space="PSUM") as pso_pool,
            ):
                for sb in range(NSBLK):
                    ssl = slice(sb * SBLK, (sb + 1) * SBLK)
                    for m in range(E // 128):
                        po = pso_pool.tile([128, SBLK], F32, name="po")
                        for h in range(HPC):
                            nc.tensor.matmul(
                                po[:],
                                wo_t[h][:, m * 128 : (m + 1) * 128],
                                ctx16[h][sb][:],
                                start=(h == 0),
                                stop=(h == HPC - 1),
                            )
                        ot = osb_pool.tile([128, SBLK], F32, tag="ot", name="ot")
                        nc.scalar.copy(ot[:], po[:])
                        nc.sync.dma_start(outT[m * 128 : (m + 1) * 128, ssl], ot[:])
            lconst_cm.__exit__(None, None, None)
            wo_cm.__exit__(None, None, None)
            ctx_cm.__exit__(None, None, None)

    nc.compile()
    return nc


def _finalize(nc, pending, rcp_pool, bcs_pool, ctx16):
    h, t, ctx_ps, den_ps = pending
    rc = rcp_pool.tile([1, SBLK], F32, tag="rc", name="rc")
    nc.vector.reciprocal_approx_fast(out=rc[0:1, :], in_=den_ps[0:1, :])
    bc_sb = bcs_pool.tile([128, SBLK], F32, tag="bc", name="bc_sb")
    nc.gpsimd.partition_broadcast(bc_sb[:], rc[0:1, :])
    nc.vector.tensor_mul(ctx16[h][t][:], ctx_ps[:], bc_sb[:])


def _rope_tables():
    inv_freq = 1.0 / (10000.0 ** (np.arange(0, D, 2, dtype=np.float64) / D))
    t = np.arange(S, dtype=np.float64)
    freqs = np.outer(t, inv_freq)  # (S, D/2)
    emb = np.concatenate([freqs, freqs], axis=-1)  # (S, D)
    cosT = np.cos(emb).T.astype(np.float32).copy()  # (D, S)
    sinT = np.sin(emb).T.astype(np.float32)
    sinTs = sinT.copy()
    sinTs[: D // 2] = -sinT[: D // 2]
    return cosT, np.ascontiguousarray(sinTs)


def _binmask():
    r = np.arange(128)[:, None]
    c = np.arange(SBLK)[None, :]
    blocks = [(r + o * 128 <= c).astype(np.float32) for o in range(4)]
    return np.ascontiguousarray(np.concatenate(blocks, axis=1))


def _numpy_fallback(x, mask, wqkv, bqkv, wout, bout):
    qkv = x @ wqkv.T + bqkv
    q, k, v = np.split(qkv, 3, axis=-1)
    q = q.reshape(B, S, H, D).transpose(0, 2, 1, 3)
    k = k.reshape(B, S, H, D).transpose(0, 2, 1, 3)
    v = v.reshape(B, S, H, D).transpose(0, 2, 1, 3)
    inv_freq = 1.0 / (10000.0 ** (np.arange(0, D, 2, dtype=np.float32) / D))
    t = np.arange(S, dtype=np.float32)
    freqs = np.outer(t, inv_freq)
    emb = np.concatenate([freqs, freqs], axis=-1)
    cos, sin = np.cos(emb), np.sin(emb)

    def rot(a):
        a1, a2 = np.split(a, 2, axis=-1)
        return np.concatenate([-a2, a1], axis=-1)

    q = q * cos + rot(q) * sin
    k = k * cos + rot(k) * sin
    scores = np.einsum("bhqd,bhkd->bhqk", q, k) * SM_SCALE
    scores = np.where(mask, -np.inf, scores)
    scores = scores - scores.max(axis=-1, keepdims=True)
    w = np.exp(scores)
    w = w / w.sum(axis=-1, keepdims=True)
    ctx = np.einsum("bhqk,bhkd->bhqd", w, v)
    ctx = ctx.transpose(0, 2, 1, 3).reshape(B, S, E)
    return (ctx @ wout.T + bout).astype(np.float32)


def kernel(x, mask, wqkv, bqkv, wout, bout, **_):
    x = np.ascontiguousarray(np.asarray(x), dtype=np.float32)
    wqkv = np.ascontiguousarray(np.asarray(wqkv), dtype=np.float32)
    bqkv = np.asarray(bqkv, dtype=np.float32)
    wout = np.ascontiguousarray(np.asarray(wout), dtype=np.float32)
    bout = np.asarray(bout, dtype=np.float32)
    mask = np.asarray(mask)

    causal = np.array_equal(mask, np.triu(np.ones((S, S), dtype=bool), k=1))
    if not causal or np.any(bqkv):
        return _numpy_fallback(x, mask, wqkv, bqkv, wout, bout)

    if "nc" not in _CACHE:
        _CACHE["nc"] = _build_nc()
    nc = _CACHE["nc"]

    cosT, sinTs = _rope_tables()
    bm = _binmask()
    ones = np.ones((128, 128), dtype=np.float32)

    in_maps = []
    for c in range(N_CORES):
        b, hg = divmod(c, GPB)
        cols = slice(hg * DPC, (hg + 1) * DPC)
        wq = wqkv[0 * E : 1 * E, :][cols, :]  # (512, E)
        wk = wqkv[1 * E : 2 * E, :][cols, :]
        wv = wqkv[2 * E : 3 * E, :][cols, :]
        in_maps.append(
            {
                "xT": np.ascontiguousarray(x[b].T),
                "wqkT": np.ascontiguousarray(np.concatenate([wq, wk], axis=0).T),
                "wvT": np.ascontiguousarray(wv.T),
                "woutT": np.ascontiguousarray(wout[:, cols].T),  # (512, E)
                "cosT": cosT,
                "sinTs": sinTs,
                "bmask": bm,
                "ones": ones,
            }
        )

    res = bass_utils.run_bass_kernel_spmd(
        nc, in_maps, core_ids=list(range(N_CORES)), **_RUN_KWARGS
    )
    _CACHE["last_results"] = res

    out = np.empty((B, S, E), dtype=np.float32)
    for b in range(B):
        acc = res.results[b * GPB]["outT"].copy()
        for g in range(1, GPB):
            acc += res.results[b * GPB + g]["outT"]
        out[b] = acc.T
    out += bout
    return out
